# revision 6
# baseline (speedup 1.0000x reference)
"""Trainium2 Bass kernel for the 3-layer LSTM scan (nn_Net_2095944040841).

Feature-on-partition redesign. Per core (batch 512/8 = 64 on the free axis):

  - Moving state tiles stack features on partitions so K-concatenated
    stationaries fuse matmuls.  Engine partition windows must start at a
    quadrant base (0/32/64/96), which dictates the row layout:
      M12 [128, CH*64] chunk buffer, one 64-col slot per tick:
        rows 0:50   h2_hat(t-2)     (cell2 block lives at base 0)
        rows 50:100 h1_hat(t-1)     (cell1 block at rows 50:100)
        rows 100:120 x(t), row 120 ones  (DMA'd once per chunk from a
                                          [21, T, 64] tensor with ones row)
      M3h [65, 64]: rows 0:50 h3_hat, row 64 ones (ring of 2).
  - Gate matmuls: 4 stacked K=121 matmuls (one per gate, cell2 cols 0:50 /
    cell1 cols 50:100 of the stationary) + 4 cell3 matmuls + 1 output
    matmul = 9/tick (vs 13 in the col-block layout).
  - Cell3's o3 gate is parked at psum rows 64:114 (base-64 window) so the
    h3 elementwise ops stay quadrant-legal; i3/g3 sit at rows 0:50.
  - Reference quirk preserved: c3 stays 0 (no f3 path) and cell2's c_prev
    is cell3's product p3 = i3*g3 from the previous timestep; p3_hat is
    dropped into UU rows 0:50 next to s1_hat rows 50:100 so one stt
    handles both cells' f*c_prev.
  - tanh-only activations (sigma via tanh trick), hat-scaled states
    (h_hat=2h, s_hat=2c) with pre-scaled weights; biases ride ones rows.
"""

import sys

sys.path.insert(0, "/opt/trn_rl_repo")

import numpy as np

import concourse.bass as bass
import concourse.tile as tile
from concourse import bacc, mybir

HID = 50
IN_DIM = 20
OUT_DIM = 8
B_FULL = 512
T_FULL = 1024
N_CORES = 8

b = 64          # batch per core
CH = 64         # x-chunk length in ticks
OB = 8          # output accumulation ticks per DMA

F32 = mybir.dt.float32
BF16 = mybir.dt.bfloat16
CDT = BF16
import ml_dtypes
NP_CDT = ml_dtypes.bfloat16

# gate row ranges in the reference 4*HID layout: i, f, g, o
GATES = {"i": slice(0, 50), "f": slice(50, 100), "g": slice(100, 150),
         "o": slice(150, 200)}
GSC = {"i": 0.5, "f": 0.5, "o": 0.5, "g": 1.0}


def prep_params(W1, b1, Wih1, Whh1, bih1, bhh1, Wih2, Whh2, bih2, bhh2,
                Wih3, Whh3, bih3, bhh3, W2, b2):
    """Host-side weight transformation. Returns {name: np.float32 array}."""
    f = np.float32
    W1, b1 = np.asarray(W1, f), np.asarray(b1, f)
    Wih1, Whh1 = np.asarray(Wih1, f), np.asarray(Whh1, f)
    Wih2, Whh2 = np.asarray(Wih2, f), np.asarray(Whh2, f)
    Wih3, Whh3 = np.asarray(Wih3, f), np.asarray(Whh3, f)
    W2, b2 = np.asarray(W2, f), np.asarray(b2, f)
    Wc1 = Wih1 @ W1                                   # [200, 20]
    bc1 = Wih1 @ b1 + np.asarray(bih1, f) + np.asarray(bhh1, f)
    b2c = np.asarray(bih2, f) + np.asarray(bhh2, f)
    b3c = np.asarray(bih3, f) + np.asarray(bhh3, f)

    out = {}
    # stacked cells1+2 stationaries, one per gate: [121, 100]
    # moving rows: 0:50 h2_hat, 50:100 h1_hat, 100:120 x, 120 ones
    # stationary cols: 0:50 cell2 gates, 50:100 cell1 gates
    for gn in "igfo":
        G, sc = GATES[gn], GSC[gn]
        w = np.zeros((121, 100), f)
        w[0:50, 0:50] = sc * 0.5 * Whh2[G].T          # cell2 rec (h2)
        w[50:100, 0:50] = sc * 0.5 * Wih2[G].T        # cell2 in  (h1)
        w[50:100, 50:100] = sc * 0.5 * Whh1[G].T      # cell1 rec (h1)
        w[100:120, 50:100] = sc * Wc1[G].T            # cell1 in  (x)
        w[120, 0:50] = sc * b2c[G]
        w[120, 50:100] = sc * bc1[G]
        out[f"w12{gn}"] = w
    # cell3 moving tile M3 [128, 64]: rows 0:50 h2_hat, 64:114 h3_hat,
    # row 120 ones.  One non-accumulating matmul per gate group:
    # w3A: (i|o) -> psum rows 0:50 / 64:114; w3B: g -> rows 0:50.
    # (no f path: c3 = 0)
    w3A = np.zeros((121, 114), f)
    w3A[0:50, 0:50] = 0.25 * Wih3[GATES["i"]].T
    w3A[0:50, 64:114] = 0.25 * Wih3[GATES["o"]].T
    w3A[64:114, 0:50] = 0.25 * Whh3[GATES["i"]].T
    w3A[64:114, 64:114] = 0.25 * Whh3[GATES["o"]].T
    w3A[120, 0:50] = 0.5 * b3c[GATES["i"]]
    w3A[120, 64:114] = 0.5 * b3c[GATES["o"]]
    w3B = np.zeros((121, 50), f)
    w3B[0:50, :] = 0.5 * Wih3[GATES["g"]].T
    w3B[64:114, :] = 0.5 * Whh3[GATES["g"]].T
    w3B[120, :] = b3c[GATES["g"]]
    out["w3A"], out["w3B"] = w3A, w3B
    # output projection reads M3 rows 64:121 (base-64 window); the
    # stationary is sliced at the same base so rows 64:121 hold weights
    w2e = np.zeros((121, OUT_DIM), f)
    w2e[64:114, :] = 0.5 * W2.T
    w2e[120, :] = b2
    out["w2e"] = w2e
    return out


def build_nc(T=T_FULL):
    """Build the Bass module for one core (SPMD across 8)."""
    nc = bacc.Bacc(None, target_bir_lowering=False)
    TANH = mybir.ActivationFunctionType.Tanh
    ADD, MUL = mybir.AluOpType.add, mybir.AluOpType.mult

    xt = nc.dram_tensor("xt", [IN_DIM + 1, T, b], CDT, kind="ExternalInput")
    wshapes = {"w12i": [121, 100], "w12g": [121, 100], "w12f": [121, 100],
               "w12o": [121, 100], "w3A": [121, 114], "w3B": [121, 50],
               "w2e": [121, OUT_DIM]}
    wd = {n: nc.dram_tensor(n, s, CDT, kind="ExternalInput")
          for n, s in wshapes.items()}
    out_d = nc.dram_tensor("out", [T, OUT_DIM, b], F32, kind="ExternalOutput")

    n_chunks = (T + CH - 1) // CH

    with tile.TileContext(nc) as tc:
        with (
            tc.tile_pool(name="weights", bufs=1) as wp,
            tc.tile_pool(name="state", bufs=1) as sp,
            tc.tile_pool(name="xs", bufs=1) as xp,
            tc.tile_pool(name="work", bufs=3) as wk,
            tc.tile_pool(name="psum", bufs=2, space="PSUM") as pp,
            tc.tile_pool(name="opsum", bufs=1, space="PSUM") as op_pool,
        ):
            wt = {}
            for n, s in wshapes.items():
                t = wp.tile(s, CDT, name=n, tag=n)
                nc.sync.dma_start(t[:], wd[n][:])
                wt[n] = t

            M12 = [xp.tile([128, CH * b], CDT, name=f"m12_{i}", tag=f"m12_{i}")
                   for i in range(2)]
            M3 = [sp.tile([128, b], CDT, name=f"m3_{i}", tag=f"m3_{i}")
                  for i in range(2)]
            UU = [sp.tile([100, b], CDT, name=f"uu_{i}", tag=f"uu_{i}")
                  for i in range(2)]
            for i in range(2):
                nc.vector.memset(M3[i][0:120, :], 0.0)
                # ones row 120 (quadrant rule bars a memset there; DMA the
                # xt ones row instead)
                nc.sync.dma_start(
                    M3[i][120:121, :],
                    xt[IN_DIM:IN_DIM + 1, 0:1, :].rearrange("p a c -> p (a c)"))
                nc.vector.memset(UU[i][0:100, :], 0.0)
            # zero h-state rows of slots 0,1 of chunk buffer 0
            nc.vector.memset(M12[0][0:100, 0:2 * b], 0.0)
            nc.sync.dma_start(
                M12[0][100:121, :].rearrange("p (t c) -> p t c", t=CH),
                xt[:, 0:CH, :])

            out_ring = [op_pool.tile([OUT_DIM, OB * b], F32, name=f"ob{i}",
                                     tag=f"ob{i}") for i in range(2)]

            for k in range(T + 3):
                run12 = k <= T
                run3 = 2 <= k <= T + 1
                run_out = 3 <= k
                cb = (k // CH) % 2
                sl = (k % CH) * b
                cb2 = ((k + 1) // CH) % 2
                sl2 = ((k + 1) % CH) * b

                if run12:
                    c = k // CH
                    if k % CH == 0 and c + 1 < n_chunks:
                        nb = (c + 1) % 2
                        nc.sync.dma_start(
                            M12[nb][100:121, :].rearrange("p (t c) -> p t c",
                                                          t=CH),
                            xt[:, (c + 1) * CH:(c + 2) * CH, :])
                    mv12 = M12[cb][0:121, sl:sl + b]
                    P = pp.tile([100, 256], F32, tag="p12")
                    for gi, gn in enumerate("igfo"):
                        nc.tensor.matmul(P[:, gi * b:(gi + 1) * b],
                                         wt[f"w12{gn}"][:], mv12,
                                         start=True, stop=True)
                    # T12a = (Ti|Tg), T12b = (Tf|To); rows 0:50 cell2,
                    # rows 50:100 cell1
                    t12a = wk.tile([100, 128], CDT, tag="t12a")
                    t12b = wk.tile([100, 128], CDT, tag="t12b")
                    nc.scalar.activation(t12a[:], P[:, 0:128], TANH)
                    nc.scalar.activation(t12b[:], P[:, 128:256], TANH)

                if run3:
                    # cell3 block for t=k-2: moving M3[k%2] = [h2(k-2);
                    # h3(k-3); 1], one non-accumulating matmul per group
                    mv3 = M3[k % 2][0:121, :]
                    P3 = pp.tile([114, 128], F32, tag="p3")
                    nc.tensor.matmul(P3[0:114, 0:b], wt["w3A"][:], mv3,
                                     start=True, stop=True)
                    nc.tensor.matmul(P3[0:50, b:2 * b], wt["w3B"][:], mv3,
                                     start=True, stop=True)
                    t3 = wk.tile([114, 128], CDT, tag="t3")
                    nc.scalar.activation(t3[0:114, :], P3[0:114, :], TANH)

                if run_out:
                    # out(t) = w2e.T @ [h3_hat(t); junk; 1] for t = k-3
                    # (reads M3 rows 64:121 written by the previous tick's
                    # h3 op, so it never stalls the PE queue)
                    t_out = k - 3
                    oslot = (t_out // OB) % 2
                    ocol = (t_out % OB) * b
                    nc.tensor.matmul(out_ring[oslot][:, ocol:ocol + b],
                                     wt["w2e"][64:121, :],
                                     M3[k % 2][64:121, :],
                                     start=True, stop=True)

                if run12:
                    # p12 = (Ti+1)*Tg
                    ppt = wk.tile([100, b], CDT, tag="pp")
                    nc.vector.scalar_tensor_tensor(
                        ppt[:], t12a[:, 0:b], 1.0, t12a[:, b:2 * b], ADD, MUL)

                if run3:
                    # p3_hat = (Ti3+1)*Tg3 -> UU rows 0:50 (cell2's c_prev)
                    nc.vector.scalar_tensor_tensor(
                        UU[(k + 1) % 2][0:50, :], t3[0:50, 0:b], 1.0,
                        t3[0:50, b:2 * b], ADD, MUL)

                if run12:
                    # a2 = (Tf+1)*[p3' ; s1']
                    aat = wk.tile([100, b], CDT, tag="aa")
                    nc.vector.scalar_tensor_tensor(
                        aat[:], t12b[:, 0:b], 1.0, UU[(k + 1) % 2][0:100, :],
                        ADD, MUL)
                    # u = 0.5*a2 + p : rows 0:50 u2_hat, rows 50:100 s1_hat
                    nc.vector.scalar_tensor_tensor(
                        UU[k % 2][0:100, :], aat[:], 0.5, ppt[:], MUL, ADD)

                if run3:
                    tc3 = wk.tile([114, b], CDT, tag="tc3")
                    nc.scalar.activation(tc3[64:114, :],
                                         UU[(k + 1) % 2][0:50, :], TANH,
                                         scale=0.5)
                if run12:
                    tc12 = wk.tile([100, b], CDT, tag="tc12")
                    nc.scalar.activation(tc12[:], UU[k % 2][0:100, :], TANH,
                                         scale=0.5)
                    if k == 0:
                        # restore p3(-1) = 0 for tick 1's a2 (u12 wrote
                        # garbage into the u2 slot)
                        nc.vector.memset(UU[0][0:50, :], 0.0)
                if run3:
                    # h3_hat = (To3+1)*tanh(p3); o3/tc3 sit at rows 64:114
                    nc.vector.scalar_tensor_tensor(
                        M3[(k + 1) % 2][64:114, :], t3[64:114, 0:b], 1.0,
                        tc3[64:114, :], ADD, MUL)
                if run12:
                    # h12: [h2_hat(k-1); h1_hat(k)] -> M12 slot k+1
                    nc.vector.scalar_tensor_tensor(
                        M12[cb2][0:100, sl2:sl2 + b],
                        t12b[:, b:2 * b], 1.0, tc12[:], ADD, MUL)
                    if k == 0:
                        # h2(-1) = 0 for tick 1's cell2
                        nc.vector.memset(M12[cb2][0:50, sl2:sl2 + b], 0.0)
                    # second copy of h2_hat(k-1) into the cell3 moving tile
                    nc.vector.scalar_tensor_tensor(
                        M3[(k + 1) % 2][0:50, :],
                        t12b[0:50, b:2 * b], 1.0, tc12[0:50, :], ADD, MUL)

                if run_out and t_out % OB == OB - 1:
                    t0 = t_out - OB + 1
                    ob_sb = wk.tile([OUT_DIM, OB * b], F32, tag="ob_sb")
                    # Scalar engine: its ~1us idle window at tick end absorbs
                    # this copy; on DVE it delays the next tick's chain-head
                    # ops (in-order queue) by up to ~700ns on flush ticks.
                    nc.scalar.copy(ob_sb[:], out_ring[oslot][:])
                    nc.sync.dma_start(
                        out_d[t0:t0 + OB, :, :].rearrange("t p c -> p t c"),
                        ob_sb[:].rearrange("p (t c) -> p t c", t=OB))
    nc.compile()
    return nc


def make_in_maps(inputs):
    x = np.asarray(inputs["x"], np.float32)          # [512, 1024, 20]
    B, T, _ = x.shape
    params = prep_params(**{k: v for k, v in inputs.items() if k != "x"})
    in_maps = []
    for c in range(N_CORES):
        xc = x[c * b:(c + 1) * b]                    # [64, T, 20]
        xtc = np.empty((IN_DIM + 1, T, b), np.float32)
        xtc[0:IN_DIM] = xc.transpose(2, 1, 0)
        xtc[IN_DIM] = 1.0                            # ones row for biases
        m = {"xt": xtc.astype(NP_CDT)}
        m.update({k: v.astype(NP_CDT) for k, v in params.items()})
        in_maps.append(m)
    return in_maps


def gather_out(res, B, T):
    out = np.empty((B, T, OUT_DIM), np.float32)
    for c in range(N_CORES):
        out[c * b:(c + 1) * b] = res.results[c]["out"].transpose(2, 0, 1)
    return out


def kernel(**inputs):
    from concourse.bass_utils import run_bass_kernel_spmd

    x = np.asarray(inputs["x"], np.float32)
    B, T, _ = x.shape
    nc = build_nc(T)
    in_maps = make_in_maps(inputs)

    res = run_bass_kernel_spmd(nc, in_maps, core_ids=list(range(N_CORES)))
    return gather_out(res, B, T)
